# revision 61
# baseline (speedup 1.0000x reference)
"""Trainium2 Bass kernel for nn_AttentionModule (GNN message passing).

kernel(**inputs) takes the FULL unsharded inputs (as produced by
setup_inputs) and returns the FULL [B, 128] float32 output.

Strategy: data-parallel over graphs across 8 NeuronCores (batch is sorted, so
each core owns a contiguous range of graphs/nodes).  Per core, graphs are
packed into blocks of 32x128-node tiles with <= 18 graph slots; all segment
reductions are local matmuls against host-built one-hot slabs.

v2 dataflow: x is loaded ONCE (feature-major xt only).  z is computed
feature-major via 4 zero-padded fc2 column-block weights, so
  y2t = xt * sigmoid(2 zT)        (sigmoid trick: (1+tanh z)x = 2 sig(2z) x)
then y2n = PE-transpose(y2t) feeds the segment-sum matmuls.  The x2=2*y2
scaling is folded into the host slab (S*2) so the final segment sum lands in
PSUM already scaled and is DMAed straight to DRAM.

  meanT = (y2t @ S2) * (1/cnt);  tGT = tanh(Wm.T @ meanT)
  dots = y2t.T @ tGT;  c8 = S2*sigma(2*dots);  outT = y2n.T-chain @ c8
"""

import sys
import numpy as np

sys.path.insert(0, "/opt/trn_rl_repo")

import ml_dtypes
from contextlib import ExitStack

import concourse.bass as bass
import concourse.bacc as bacc
import concourse.tile as tile
from concourse import mybir
from concourse.bass_utils import run_bass_kernel_spmd

BF = mybir.dt.bfloat16
F32 = mybir.dt.float32
ALU = mybir.AluOpType
ACTF = mybir.ActivationFunctionType
NPBF = ml_dtypes.bfloat16

# engine-balance knobs (NOTE: GPSIMD/Pool cannot touch PSUM — only sbuf->sbuf
# work is eligible for Pool: the y2t mult and the c8 mult)
RELU_ACT_OF4 = 0       # of every 4 relu ops, how many on ACT (rest DVE)
C8_ENG = "dve"         # c8 = slab*sd mult: "dve" | "pool"
MULT_POOL_W = 1344     # columns (of 2048 per g16) of the y2t mult on Pool
BUFS = {"xt": 8, "sl": 3, "h": 6, "sig": 5, "y2t": 4, "y2n": 4}

NCORES = 8
D = 128
TBLK = 32          # 128-node tiles per block
GBLK = 17          # graph slots per block (data max is 17)


class Cfg:
    def __init__(self, NB, TBLK=TBLK, GBLK=GBLK):
        self.NB = NB
        self.TBLK = TBLK
        self.GBLK = GBLK
        self.NTILES = NB * TBLK
        self.NNODES = self.NTILES * 128


# ---------------------------------------------------------------------------
# device program
# ---------------------------------------------------------------------------

def declare_io(nc, cfg):
    NB, GBLK = cfg.NB, cfg.GBLK
    d = {}
    d["xt"] = nc.dram_tensor("xt", [128, cfg.NTILES * 128], BF, kind="ExternalInput").ap()
    d["sl"] = nc.dram_tensor("sl", [128, cfg.NTILES * GBLK], BF, kind="ExternalInput").ap()
    d["recip"] = nc.dram_tensor("recip", [128, NB * GBLK], F32, kind="ExternalInput").ap()
    d["fc1t"] = nc.dram_tensor("fc1t", [128, 32], BF, kind="ExternalInput").ap()
    d["fc2e"] = nc.dram_tensor("fc2e", [128, 512], BF, kind="ExternalInput").ap()
    d["wm"] = nc.dram_tensor("wm", [128, 128], F32, kind="ExternalInput").ap()
    d["b1"] = nc.dram_tensor("b1", [128, 1], F32, kind="ExternalInput").ap()
    d["ident"] = nc.dram_tensor("ident", [128, 128], BF, kind="ExternalInput").ap()
    d["outT"] = nc.dram_tensor("outT", [128, NB * GBLK], F32, kind="ExternalOutput").ap()
    return d


def build(tc, io, cfg):
    nc = tc.nc
    NB, TBLK, GBLK = cfg.NB, cfg.TBLK, cfg.GBLK
    assert TBLK == 32

    with ExitStack() as ctx:
        ep = ctx.enter_context

        consts = ep(tc.tile_pool(name="consts", bufs=1))
        # earliest-needed consts first so the g16=0 front-end starts ASAP;
        # the first x blocks are prefetched before the bulky consts
        fc1t = consts.tile([128, 32], BF, tag="fc1t")
        nc.sync.dma_start(fc1t[:], io["fc1t"])

        xtp = ep(tc.tile_pool(name="xt", bufs=BUFS["xt"]))
        slp = ep(tc.tile_pool(name="sl", bufs=BUFS["sl"]))
        hp = ep(tc.tile_pool(name="h", bufs=BUFS["h"]))
        sigp = ep(tc.tile_pool(name="sig", bufs=BUFS["sig"]))
        y2tp = ep(tc.tile_pool(name="y2t", bufs=BUFS["y2t"]))
        y2np = ep(tc.tile_pool(name="y2n", bufs=BUFS["y2n"]))
        mtp = ep(tc.tile_pool(name="mt", bufs=3))
        tgp = ep(tc.tile_pool(name="tg", bufs=3))
        sdp = ep(tc.tile_pool(name="sd", bufs=3))
        cp = ep(tc.tile_pool(name="c8", bufs=3))
        outp = ep(tc.tile_pool(name="osb", bufs=2))

        def load_xt(blk, chunks=1):
            nbase = blk * TBLK * 128
            xt = xtp.tile([128, TBLK * 128], BF, tag="xt")
            w = TBLK * 128 // chunks
            for ci in range(chunks):
                nc.sync.dma_start(xt[:, ci * w:(ci + 1) * w],
                                  io["xt"][:, nbase + ci * w:nbase + (ci + 1) * w])
            return xt

        def load_sl(blk):
            # slab superblock for blocks blk, blk+1; issued on the ACT ring
            ssb2 = slp.tile([128, 2 * TBLK * GBLK], BF, tag="sl")
            hi = min(blk + 2, NB)
            nc.scalar.dma_start(
                ssb2[:, 0:(hi - blk) * TBLK * GBLK],
                io["sl"][:, blk * TBLK * GBLK:hi * TBLK * GBLK])
            return ssb2

        pre_xt = [load_xt(0, chunks=2)]
        b1c = consts.tile([128, 1], F32, tag="b1c")
        nc.sync.dma_start(b1c[:], io["b1"])
        if NB > 1:
            pre_xt.append(load_xt(1))
        pre_sl = load_sl(0)

        fc2e = consts.tile([128, 512], BF, tag="fc2e")
        nc.sync.dma_start(fc2e[:], io["fc2e"])
        ident = consts.tile([128, 128], BF, tag="ident")
        nc.sync.dma_start(ident[:], io["ident"])
        wm = consts.tile([128, 128], F32, tag="wm")
        nc.sync.dma_start(wm[:], io["wm"])
        recip = consts.tile([128, NB * GBLK], F32, tag="recip")
        nc.sync.dma_start(recip[:], io["recip"])

        # PSUM pools — 8 banks: ph 1 (f32), pz 2x2 ([128,1024] f32),
        # pyt 1 (bf16), pmf 1 (f32), pd 1 (f32)
        php = ep(tc.tile_pool(name="ph", bufs=1, space="PSUM"))
        pzp = ep(tc.tile_pool(name="pz", bufs=2, space="PSUM"))
        pytp = ep(tc.tile_pool(name="pyt", bufs=1, space="PSUM"))
        pmfp = ep(tc.tile_pool(name="pmf", bufs=1, space="PSUM"))
        pdp = ep(tc.tile_pool(name="pd", bufs=1, space="PSUM"))

        GT = NB * 2
        gst = [None] * GT     # per-g16 pipeline state
        bst = [None] * NB     # per-block state

        def sA(gi):
            """block allocs/DMAs + fc1 (PE)"""
            blk, g16 = divmod(gi, 2)
            if g16 == 0:
                xt = pre_xt[blk] if blk < len(pre_xt) else load_xt(blk)
                if blk % 2 == 0:
                    ssb2 = pre_sl if blk == 0 else load_sl(blk)
                else:
                    ssb2 = bst[blk - 1]["ssb2"]
                y2t = y2tp.tile([128, TBLK * 128], BF, tag="y2t")
                y2n = y2np.tile([128, TBLK * 128], BF, tag="y2n")
                # pmf bank: mean [0:G], fin [32:32+G], tG [96:96+G]
                pmf = pmfp.tile([128, 512], F32, tag="pmf")
                bst[blk] = {
                    "xt": xt, "ssb2": ssb2,
                    "ssb": ssb2[:, (blk % 2) * TBLK * GBLK:
                                (blk % 2 + 1) * TBLK * GBLK],
                    "y2t": y2t, "y2n": y2n, "pmf": pmf,
                }
            xt = bst[blk]["xt"]
            gb = g16 * 2048
            ph = php.tile([128, 512], F32, tag="ph")
            # one matmul per column-group j covers 4 tiles (s 0..3) via a
            # strided rhs AP; h lands packed as ph[32j+u, s*128+k]
            xtg = xt[:, gb:gb + 2048].rearrange(
                "p (s j k) -> p j s k", s=4, j=4, k=128)
            for j in range(4):
                nc.tensor.matmul(
                    ph[32 * j:32 * j + 32, 0:512],
                    fc1t[:], xtg[:, j],
                    start=True, stop=True, tile_position=(0, 32 * j))
            gst[gi] = {"ph": ph}

        def sF(gi):
            """relu -> h16 (DVE; emitted late so it doesn't head-of-line
            block earlier DVE work behind the fresh fc1)"""
            ph = gst[gi]["ph"]
            h16 = hp.tile([128, 512], BF, tag="h")
            if gi % 4 < RELU_ACT_OF4:
                nc.scalar.activation(h16[:], ph[:], ACTF.Relu, bias=b1c[:])
            else:
                nc.vector.tensor_scalar(h16[:], ph[:], b1c[:], 0.0,
                                        op0=ALU.add, op1=ALU.max)
            gst[gi]["h16"] = h16

        def sZ(gi, half):
            """zT half (8 small matmuls into a [128,1024] psum chunk) +
            one sigmoid; 2-deep pz rotation keeps the stage decoupled"""
            h16 = gst[gi]["h16"]
            if half == 0:
                sigT = sigp.tile([128, 2048], BF, tag="sig")
                gst[gi]["sigT"] = sigT
            sigT = gst[gi]["sigT"]
            sview = sigT.rearrange("p (s j k) -> p s j k", s=4, j=4, k=128)
            pz = pzp.tile([128, 1024], F32, tag="pz")
            for jj in range(2):
                j = half * 2 + jj
                for s in range(4):
                    nc.tensor.matmul(
                        pz[:, s * 256 + jj * 128:s * 256 + jj * 128 + 128],
                        fc2e[:, j * 128:(j + 1) * 128],
                        h16[:, s * 128:(s + 1) * 128],
                        start=True, stop=True)
            # sigmoid psum->sbuf, out strided to node order
            nc.scalar.activation(
                sview[:, :, 2 * half:2 * half + 2, :],
                pz[:].rearrange("p (s j k) -> p s j k", s=4, j=2, k=128),
                ACTF.Sigmoid, scale=2.0)

        def sE(gi):
            """y2t = xt * sigT (DVE + Pool)"""
            blk, g16 = divmod(gi, 2)
            b = bst[blk]
            xt, y2t = b["xt"], b["y2t"]
            sigT = gst[gi]["sigT"]
            gb = g16 * 2048
            # pool takes the last MULT_POOL_W cols (issued first, it's slow);
            # DVE covers the rest in two chunks
            pw = MULT_POOL_W
            bounds = [(2048 - pw, 2048, True),
                      (0, (2048 - pw) // 2, False),
                      ((2048 - pw) // 2, 2048 - pw, False)]
            for lo, hi, on_pool in bounds:
                if hi <= lo:
                    continue
                meng = nc.gpsimd if on_pool else nc.vector
                meng.tensor_tensor(
                    y2t[:, gb + lo:gb + hi],
                    xt[:, gb + lo:gb + hi],
                    sigT[:, lo:hi], op=ALU.mult)

        def sC(gi):
            """transpose y2t -> y2n (PE + DVE copies), mean accumulation"""
            blk, g16 = divmod(gi, 2)
            b = bst[blk]
            y2t, y2n = b["y2t"], b["y2n"]
            for t8 in range(2):
                pyt = pytp.tile([128, 1024], BF, tag="pyt")
                for k in range(8):
                    t = g16 * 16 + t8 * 8 + k
                    nc.tensor.transpose(
                        pyt[:, k * 128:(k + 1) * 128],
                        y2t[:, t * 128:(t + 1) * 128], ident[:])
                c0 = (g16 * 16 + t8 * 8) * 128
                nc.vector.tensor_copy(y2n[:, c0:c0 + 1024], pyt[:])
            for k16 in range(16):
                t = g16 * 16 + k16
                nc.tensor.matmul(
                    b["pmf"][:, 0:GBLK],
                    y2n[:, t * 128:(t + 1) * 128],
                    b["ssb"][:, t * GBLK:(t + 1) * GBLK],
                    start=(t == 0), stop=(t == TBLK - 1), skip_group_check=True)

        def tail(blk):
            b = bst[blk]
            pmf, y2t, y2n, ssb = b["pmf"], b["y2t"], b["y2n"], b["ssb"]
            meant = mtp.tile([128, GBLK], F32, tag="mt")
            nc.vector.tensor_tensor(
                meant[:], pmf[:, 0:GBLK],
                recip[:, blk * GBLK:(blk + 1) * GBLK], op=ALU.mult)
            nc.tensor.matmul(pmf[:, 96:96 + GBLK], wm[:], meant[:],
                             start=True, stop=True, skip_group_check=True)
            tgt = tgp.tile([128, GBLK], BF, tag="tg")
            nc.scalar.activation(tgt[:], pmf[:, 96:96 + GBLK], ACTF.Tanh)
            DG = 16   # dots tiles per psum bank
            for tg in range(TBLK // DG):
                pd = pdp.tile([128, DG * GBLK], F32, tag="pd")
                for k in range(DG):
                    t = tg * DG + k
                    nc.tensor.matmul(
                        pd[:, k * GBLK:(k + 1) * GBLK],
                        y2t[:, t * 128:(t + 1) * 128], tgt[:],
                        start=True, stop=True)
                sd = sdp.tile([128, DG * GBLK], BF, tag="sd")
                nc.scalar.activation(sd[:], pd[:], ACTF.Sigmoid, scale=2.0)
                c8 = cp.tile([128, DG * GBLK], BF, tag="c8")
                ceng = nc.gpsimd if C8_ENG == "pool" else nc.vector
                ceng.tensor_tensor(
                    c8[:], ssb[:, tg * DG * GBLK:(tg + 1) * DG * GBLK], sd[:],
                    op=ALU.mult)
                for k in range(DG):
                    t = tg * DG + k
                    nc.tensor.matmul(
                        pmf[:, 32:32 + GBLK],
                        y2n[:, t * 128:(t + 1) * 128],
                        c8[:, k * GBLK:(k + 1) * GBLK],
                        start=(t == 0), stop=(t == TBLK - 1),
                        skip_group_check=True)
            # slab carries the x2=2*y2 scale; copy then DMA on the ACT ring
            osb = outp.tile([128, GBLK], F32, tag="osb")
            nc.vector.tensor_copy(osb[:], pmf[:, 32:32 + GBLK])
            nc.scalar.dma_start(io["outT"][:, blk * GBLK:(blk + 1) * GBLK],
                                osb[:])

        # software pipeline, skewed so each in-order engine always has ready
        # work queued: fc1(g) | zTa+sig(g-1) | T/copies/mean(g-2) |
        # zTb+sig(g-1) | mult(g-1) | relu(g) | tail
        for it in range(GT + 2):
            if it < GT:
                sA(it)
            if 1 <= it <= GT:
                sZ(it - 1, 0)
            if it >= 2:
                sC(it - 2)
            if 1 <= it <= GT:
                sZ(it - 1, 1)
                sE(it - 1)
            if it < GT:
                sF(it)
            if it >= 2 and (it - 2) % 2 == 1:
                tail((it - 2) // 2)


# ---------------------------------------------------------------------------
# host-side prep / unshard
# ---------------------------------------------------------------------------

def plan_shards(batch_i32, B, ncores, tblk=TBLK, gblk=GBLK):
    cnt = np.bincount(batch_i32, minlength=B).astype(np.int64)
    starts = np.concatenate([[0], np.cumsum(cnt)])
    N = int(starts[-1])
    bounds = [0]
    for c in range(1, ncores):
        target = N * c // ncores
        g = int(np.searchsorted(starts, target))
        g = max(bounds[-1], min(g, B))
        bounds.append(g)
    bounds.append(B)
    cap = tblk * 128
    plans = []
    for c in range(ncores):
        glo, ghi = bounds[c], bounds[c + 1]
        blocks, cur, cur_nodes = [], [], 0
        for g in range(glo, ghi):
            n_g = int(cnt[g])
            assert n_g <= cap, f"graph {g} has {n_g} nodes > block capacity"
            if cur and (cur_nodes + n_g > cap or len(cur) >= gblk):
                blocks.append(cur)
                cur, cur_nodes = [], 0
            cur.append((g, int(starts[g]), n_g))
            cur_nodes += n_g
        if cur:
            blocks.append(cur)
        plans.append(blocks)
    NB = max(len(p) for p in plans)
    return plans, NB


def prep_core(x, plan, cfg):
    NB, TBLKc, GBLKc = cfg.NB, cfg.TBLK, cfg.GBLK
    xs = np.zeros((cfg.NNODES, D), np.float32)
    sl = np.zeros((cfg.NTILES * 128, GBLKc), NPBF)
    recip = np.zeros((NB, GBLKc), np.float32)
    meta = []
    for bi, blkg in enumerate(plan):
        pos = bi * TBLKc * 128
        for slot, (g, s, n_g) in enumerate(blkg):
            xs[pos:pos + n_g] = x[s:s + n_g]
            sl[pos:pos + n_g, slot] = NPBF(2.0)   # x2 = 2*y2 folded here
            recip[bi, slot] = 1.0 / max(n_g, 1)
            meta.append((bi, slot, g))
            pos += n_g
    xt = np.ascontiguousarray(xs.astype(NPBF).T)
    sl_packed = np.ascontiguousarray(
        sl.reshape(cfg.NTILES, 128, GBLKc).transpose(1, 0, 2).reshape(128, cfg.NTILES * GBLKc))
    recip_b = np.ascontiguousarray(
        np.broadcast_to(recip.reshape(1, NB * GBLKc), (128, NB * GBLKc)))
    return {"xt": xt, "sl": sl_packed, "recip": recip_b}, meta


def prep_consts(Wm, fc1_w, fc1_b, fc2_w, fc2_b):
    assert np.allclose(np.asarray(fc2_b, np.float32), 0.0), \
        "nonzero fc2_b not supported by this kernel build"
    fc1t = np.ascontiguousarray(np.asarray(fc1_w, np.float32).T.astype(NPBF))
    # fc2e[:, j*128:(j+1)*128] has fc2e[32j+u, f] = fc2_w[f, u], zero elsewhere
    fc2e = np.zeros((128, 512), NPBF)
    f2 = np.asarray(fc2_w, np.float32).astype(NPBF)   # [128 f, 32 u]
    for j in range(4):
        fc2e[32 * j:32 * j + 32, j * 128:(j + 1) * 128] = f2.T
    b1 = np.tile(np.asarray(fc1_b, np.float32), 4).reshape(128, 1)
    wm = np.ascontiguousarray(np.asarray(Wm, np.float32))
    ident = np.eye(128, dtype=NPBF)
    return {"fc1t": fc1t, "fc2e": fc2e, "wm": wm,
            "b1": np.ascontiguousarray(b1), "ident": ident}


def unshard(outTs, metas, B, cfg):
    out = np.zeros((B, D), np.float32)
    for outT, meta in zip(outTs, metas):
        cols = [bi * cfg.GBLK + slot for (bi, slot, g) in meta]
        gs = [g for (bi, slot, g) in meta]
        out[gs] = outT[:, cols].T
    return out


# ---------------------------------------------------------------------------
# top-level entry
# ---------------------------------------------------------------------------

_CACHE = {}


def _get_program(NB):
    key = (NB, TBLK, GBLK)
    if key not in _CACHE:
        nc = bacc.Bacc("TRN2", target_bir_lowering=False, debug=False,
                       num_devices=NCORES)
        cfg = Cfg(NB)
        io = declare_io(nc, cfg)
        with tile.TileContext(nc) as tc:
            build(tc, io, cfg)
        nc.compile()
        _CACHE[key] = (nc, cfg)
    return _CACHE[key]


def _run(inputs, trace=False):
    x = np.asarray(inputs["x"], np.float32)
    batch = np.asarray(inputs["batch"]).astype(np.int32)
    B = int(np.asarray(inputs["size"]))
    plans, NB = plan_shards(batch, B, NCORES)
    nc, cfg = _get_program(NB)
    consts = prep_consts(inputs["Wm"], inputs["fc1_w"], inputs["fc1_b"],
                         inputs["fc2_w"], inputs["fc2_b"])
    in_maps, metas = [], []
    for c in range(NCORES):
        core_in, meta = prep_core(x, plans[c], cfg)
        core_in.update(consts)
        in_maps.append(core_in)
        metas.append(meta)
    res = run_bass_kernel_spmd(nc, in_maps, core_ids=list(range(NCORES)),
                               trace=trace)
    outTs = [res.results[c]["outT"] for c in range(NCORES)]
    out = unshard(outTs, metas, B, cfg)
    return out, res


def kernel(**inputs):
    out, _ = _run(inputs, trace=False)
    return out


# revision 63
# speedup vs baseline: 1.0144x; 1.0144x over previous
"""Trainium2 Bass kernel for nn_AttentionModule (GNN message passing).

kernel(**inputs) takes the FULL unsharded inputs (as produced by
setup_inputs) and returns the FULL [B, 128] float32 output.

Strategy: data-parallel over graphs across 8 NeuronCores (batch is sorted, so
each core owns a contiguous range of graphs/nodes).  Per core, graphs are
packed into blocks of 32x128-node tiles with <= 18 graph slots; all segment
reductions are local matmuls against host-built one-hot slabs.

v2 dataflow: x is loaded ONCE (feature-major xt only).  z is computed
feature-major via 4 zero-padded fc2 column-block weights, so
  y2t = xt * sigmoid(2 zT)        (sigmoid trick: (1+tanh z)x = 2 sig(2z) x)
then y2n = PE-transpose(y2t) feeds the segment-sum matmuls.  The x2=2*y2
scaling is folded into the host slab (S*2) so the final segment sum lands in
PSUM already scaled and is DMAed straight to DRAM.

  meanT = (y2t @ S2) * (1/cnt);  tGT = tanh(Wm.T @ meanT)
  dots = y2t.T @ tGT;  c8 = S2*sigma(2*dots);  outT = y2n.T-chain @ c8
"""

import sys
import numpy as np

sys.path.insert(0, "/opt/trn_rl_repo")

import ml_dtypes
from contextlib import ExitStack

import concourse.bass as bass
import concourse.bacc as bacc
import concourse.tile as tile
from concourse import mybir
from concourse.bass_utils import run_bass_kernel_spmd

BF = mybir.dt.bfloat16
F32 = mybir.dt.float32
ALU = mybir.AluOpType
ACTF = mybir.ActivationFunctionType
NPBF = ml_dtypes.bfloat16

# engine-balance knobs (NOTE: GPSIMD/Pool cannot touch PSUM — only sbuf->sbuf
# work is eligible for Pool: the y2t mult and the c8 mult)
RELU_ACT_OF4 = 0       # of every 4 relu ops, how many on ACT (rest DVE)
C8_ENG = "dve"         # c8 = slab*sd mult: "dve" | "pool"
MULT_POOL_W = 1344     # columns (of 2048 per g16) of the y2t mult on Pool
BUFS = {"xt": 8, "sl": 3, "h": 6, "sig": 5, "y2t": 4, "y2n": 4}

NCORES = 8
D = 128
TBLK = 32          # 128-node tiles per block
GBLK = 18          # graph slots per block (data max is 17)


class Cfg:
    def __init__(self, NB, TBLK=TBLK, GBLK=GBLK):
        self.NB = NB
        self.TBLK = TBLK
        self.GBLK = GBLK
        self.NTILES = NB * TBLK
        self.NNODES = self.NTILES * 128


# ---------------------------------------------------------------------------
# device program
# ---------------------------------------------------------------------------

def declare_io(nc, cfg):
    NB, GBLK = cfg.NB, cfg.GBLK
    d = {}
    d["xt"] = nc.dram_tensor("xt", [128, cfg.NTILES * 128], BF, kind="ExternalInput").ap()
    d["sl"] = nc.dram_tensor("sl", [128, cfg.NTILES * GBLK], BF, kind="ExternalInput").ap()
    d["recip"] = nc.dram_tensor("recip", [128, NB * GBLK], F32, kind="ExternalInput").ap()
    d["fc1t"] = nc.dram_tensor("fc1t", [128, 32], BF, kind="ExternalInput").ap()
    d["fc2e"] = nc.dram_tensor("fc2e", [128, 512], BF, kind="ExternalInput").ap()
    d["wm"] = nc.dram_tensor("wm", [128, 128], F32, kind="ExternalInput").ap()
    d["b1"] = nc.dram_tensor("b1", [128, 1], F32, kind="ExternalInput").ap()
    d["ident"] = nc.dram_tensor("ident", [128, 128], BF, kind="ExternalInput").ap()
    d["outT"] = nc.dram_tensor("outT", [128, NB * GBLK], F32, kind="ExternalOutput").ap()
    return d


def build(tc, io, cfg):
    nc = tc.nc
    NB, TBLK, GBLK = cfg.NB, cfg.TBLK, cfg.GBLK
    assert TBLK == 32

    with ExitStack() as ctx:
        ep = ctx.enter_context

        consts = ep(tc.tile_pool(name="consts", bufs=1))
        # earliest-needed consts first so the g16=0 front-end starts ASAP;
        # the first x blocks are prefetched before the bulky consts
        fc1t = consts.tile([128, 32], BF, tag="fc1t")
        nc.sync.dma_start(fc1t[:], io["fc1t"])

        xtp = ep(tc.tile_pool(name="xt", bufs=BUFS["xt"]))
        slp = ep(tc.tile_pool(name="sl", bufs=BUFS["sl"]))
        hp = ep(tc.tile_pool(name="h", bufs=BUFS["h"]))
        sigp = ep(tc.tile_pool(name="sig", bufs=BUFS["sig"]))
        y2tp = ep(tc.tile_pool(name="y2t", bufs=BUFS["y2t"]))
        y2np = ep(tc.tile_pool(name="y2n", bufs=BUFS["y2n"]))
        mtp = ep(tc.tile_pool(name="mt", bufs=3))
        tgp = ep(tc.tile_pool(name="tg", bufs=3))
        sdp = ep(tc.tile_pool(name="sd", bufs=3))
        cp = ep(tc.tile_pool(name="c8", bufs=3))
        outp = ep(tc.tile_pool(name="osb", bufs=2))

        def load_xt(blk, chunks=1):
            nbase = blk * TBLK * 128
            xt = xtp.tile([128, TBLK * 128], BF, tag="xt")
            w = TBLK * 128 // chunks
            for ci in range(chunks):
                nc.sync.dma_start(xt[:, ci * w:(ci + 1) * w],
                                  io["xt"][:, nbase + ci * w:nbase + (ci + 1) * w])
            return xt

        def load_sl(blk):
            # slab superblock for blocks blk, blk+1; issued on the ACT ring
            ssb2 = slp.tile([128, 2 * TBLK * GBLK], BF, tag="sl")
            hi = min(blk + 2, NB)
            nc.scalar.dma_start(
                ssb2[:, 0:(hi - blk) * TBLK * GBLK],
                io["sl"][:, blk * TBLK * GBLK:hi * TBLK * GBLK])
            return ssb2

        pre_xt = [load_xt(0, chunks=2)]
        b1c = consts.tile([128, 1], F32, tag="b1c")
        nc.sync.dma_start(b1c[:], io["b1"])
        # fc2e/ident land before xt block-1's long transfer: the first zT and
        # transposes need them at ~5us, xt1 isn't needed until ~10us
        fc2e = consts.tile([128, 512], BF, tag="fc2e")
        nc.sync.dma_start(fc2e[:], io["fc2e"])
        ident = consts.tile([128, 128], BF, tag="ident")
        nc.sync.dma_start(ident[:], io["ident"])
        if NB > 1:
            pre_xt.append(load_xt(1))
        pre_sl = load_sl(0)

        wm = consts.tile([128, 128], F32, tag="wm")
        nc.sync.dma_start(wm[:], io["wm"])
        recip = consts.tile([128, NB * GBLK], F32, tag="recip")
        nc.sync.dma_start(recip[:], io["recip"])

        # PSUM pools — 8 banks: ph 1 (f32), pz 2x2 ([128,1024] f32),
        # pyt 1 (bf16), pmf 1 (f32), pd 1 (f32)
        php = ep(tc.tile_pool(name="ph", bufs=1, space="PSUM"))
        pzp = ep(tc.tile_pool(name="pz", bufs=2, space="PSUM"))
        pytp = ep(tc.tile_pool(name="pyt", bufs=1, space="PSUM"))
        pmfp = ep(tc.tile_pool(name="pmf", bufs=1, space="PSUM"))
        pdp = ep(tc.tile_pool(name="pd", bufs=1, space="PSUM"))

        GT = NB * 2
        gst = [None] * GT     # per-g16 pipeline state
        bst = [None] * NB     # per-block state

        def sA(gi):
            """block allocs/DMAs + fc1 (PE)"""
            blk, g16 = divmod(gi, 2)
            if g16 == 0:
                xt = pre_xt[blk] if blk < len(pre_xt) else load_xt(blk)
                if blk % 2 == 0:
                    ssb2 = pre_sl if blk == 0 else load_sl(blk)
                else:
                    ssb2 = bst[blk - 1]["ssb2"]
                y2t = y2tp.tile([128, TBLK * 128], BF, tag="y2t")
                y2n = y2np.tile([128, TBLK * 128], BF, tag="y2n")
                # pmf bank: mean [0:G], fin [32:32+G], tG [96:96+G]
                pmf = pmfp.tile([128, 512], F32, tag="pmf")
                bst[blk] = {
                    "xt": xt, "ssb2": ssb2,
                    "ssb": ssb2[:, (blk % 2) * TBLK * GBLK:
                                (blk % 2 + 1) * TBLK * GBLK],
                    "y2t": y2t, "y2n": y2n, "pmf": pmf,
                }
            xt = bst[blk]["xt"]
            gb = g16 * 2048
            ph = php.tile([128, 512], F32, tag="ph")
            # one matmul per column-group j covers 4 tiles (s 0..3) via a
            # strided rhs AP; h lands packed as ph[32j+u, s*128+k]
            xtg = xt[:, gb:gb + 2048].rearrange(
                "p (s j k) -> p j s k", s=4, j=4, k=128)
            for j in range(4):
                nc.tensor.matmul(
                    ph[32 * j:32 * j + 32, 0:512],
                    fc1t[:], xtg[:, j],
                    start=True, stop=True, tile_position=(0, 32 * j))
            gst[gi] = {"ph": ph}

        def sF(gi):
            """relu -> h16 (DVE; emitted late so it doesn't head-of-line
            block earlier DVE work behind the fresh fc1)"""
            ph = gst[gi]["ph"]
            h16 = hp.tile([128, 512], BF, tag="h")
            if gi % 4 < RELU_ACT_OF4:
                nc.scalar.activation(h16[:], ph[:], ACTF.Relu, bias=b1c[:])
            else:
                nc.vector.tensor_scalar(h16[:], ph[:], b1c[:], 0.0,
                                        op0=ALU.add, op1=ALU.max)
            gst[gi]["h16"] = h16

        def sZ(gi, half):
            """zT half (8 small matmuls into a [128,1024] psum chunk) +
            one sigmoid; 2-deep pz rotation keeps the stage decoupled"""
            h16 = gst[gi]["h16"]
            if half == 0:
                sigT = sigp.tile([128, 2048], BF, tag="sig")
                gst[gi]["sigT"] = sigT
            sigT = gst[gi]["sigT"]
            sview = sigT.rearrange("p (s j k) -> p s j k", s=4, j=4, k=128)
            pz = pzp.tile([128, 1024], F32, tag="pz")
            for jj in range(2):
                j = half * 2 + jj
                for s in range(4):
                    nc.tensor.matmul(
                        pz[:, s * 256 + jj * 128:s * 256 + jj * 128 + 128],
                        fc2e[:, j * 128:(j + 1) * 128],
                        h16[:, s * 128:(s + 1) * 128],
                        start=True, stop=True)
            # sigmoid psum->sbuf, out strided to node order
            nc.scalar.activation(
                sview[:, :, 2 * half:2 * half + 2, :],
                pz[:].rearrange("p (s j k) -> p s j k", s=4, j=2, k=128),
                ACTF.Sigmoid, scale=2.0)

        def sE(gi):
            """y2t = xt * sigT (DVE + Pool)"""
            blk, g16 = divmod(gi, 2)
            b = bst[blk]
            xt, y2t = b["xt"], b["y2t"]
            sigT = gst[gi]["sigT"]
            gb = g16 * 2048
            # pool takes the last MULT_POOL_W cols (issued first, it's slow);
            # DVE covers the rest in two chunks
            pw = MULT_POOL_W
            bounds = [(2048 - pw, 2048, True),
                      (0, (2048 - pw) // 2, False),
                      ((2048 - pw) // 2, 2048 - pw, False)]
            for lo, hi, on_pool in bounds:
                if hi <= lo:
                    continue
                meng = nc.gpsimd if on_pool else nc.vector
                meng.tensor_tensor(
                    y2t[:, gb + lo:gb + hi],
                    xt[:, gb + lo:gb + hi],
                    sigT[:, lo:hi], op=ALU.mult)

        def sC(gi):
            """transpose y2t -> y2n (PE + DVE copies), mean accumulation"""
            blk, g16 = divmod(gi, 2)
            b = bst[blk]
            y2t, y2n = b["y2t"], b["y2n"]
            for t8 in range(2):
                pyt = pytp.tile([128, 1024], BF, tag="pyt")
                for k in range(8):
                    t = g16 * 16 + t8 * 8 + k
                    nc.tensor.transpose(
                        pyt[:, k * 128:(k + 1) * 128],
                        y2t[:, t * 128:(t + 1) * 128], ident[:])
                c0 = (g16 * 16 + t8 * 8) * 128
                nc.vector.tensor_copy(y2n[:, c0:c0 + 1024], pyt[:])
            for k16 in range(16):
                t = g16 * 16 + k16
                nc.tensor.matmul(
                    b["pmf"][:, 0:GBLK],
                    y2n[:, t * 128:(t + 1) * 128],
                    b["ssb"][:, t * GBLK:(t + 1) * GBLK],
                    start=(t == 0), stop=(t == TBLK - 1), skip_group_check=True)

        def tail(blk):
            b = bst[blk]
            pmf, y2t, y2n, ssb = b["pmf"], b["y2t"], b["y2n"], b["ssb"]
            meant = mtp.tile([128, GBLK], F32, tag="mt")
            nc.vector.tensor_tensor(
                meant[:], pmf[:, 0:GBLK],
                recip[:, blk * GBLK:(blk + 1) * GBLK], op=ALU.mult)
            nc.tensor.matmul(pmf[:, 96:96 + GBLK], wm[:], meant[:],
                             start=True, stop=True, skip_group_check=True)
            tgt = tgp.tile([128, GBLK], BF, tag="tg")
            nc.scalar.activation(tgt[:], pmf[:, 96:96 + GBLK], ACTF.Tanh)
            DG = 16   # dots tiles per psum bank
            for tg in range(TBLK // DG):
                pd = pdp.tile([128, DG * GBLK], F32, tag="pd")
                for k in range(DG):
                    t = tg * DG + k
                    nc.tensor.matmul(
                        pd[:, k * GBLK:(k + 1) * GBLK],
                        y2t[:, t * 128:(t + 1) * 128], tgt[:],
                        start=True, stop=True)
                sd = sdp.tile([128, DG * GBLK], BF, tag="sd")
                nc.scalar.activation(sd[:], pd[:], ACTF.Sigmoid, scale=2.0)
                c8 = cp.tile([128, DG * GBLK], BF, tag="c8")
                ceng = nc.gpsimd if C8_ENG == "pool" else nc.vector
                ceng.tensor_tensor(
                    c8[:], ssb[:, tg * DG * GBLK:(tg + 1) * DG * GBLK], sd[:],
                    op=ALU.mult)
                for k in range(DG):
                    t = tg * DG + k
                    nc.tensor.matmul(
                        pmf[:, 32:32 + GBLK],
                        y2n[:, t * 128:(t + 1) * 128],
                        c8[:, k * GBLK:(k + 1) * GBLK],
                        start=(t == 0), stop=(t == TBLK - 1),
                        skip_group_check=True)
            # slab carries the x2=2*y2 scale; copy then DMA on the ACT ring
            osb = outp.tile([128, GBLK], F32, tag="osb")
            nc.vector.tensor_copy(osb[:], pmf[:, 32:32 + GBLK])
            nc.scalar.dma_start(io["outT"][:, blk * GBLK:(blk + 1) * GBLK],
                                osb[:])

        # software pipeline, skewed so each in-order engine always has ready
        # work queued: fc1(g) | zTa+sig(g-1) | T/copies/mean(g-2) |
        # zTb+sig(g-1) | mult(g-1) | relu(g) | tail
        for it in range(GT + 2):
            if it < GT:
                sA(it)
            if 1 <= it <= GT:
                sZ(it - 1, 0)
            if it >= 2:
                sC(it - 2)
            if 1 <= it <= GT:
                sZ(it - 1, 1)
                sE(it - 1)
            if it < GT:
                sF(it)
            if it >= 2 and (it - 2) % 2 == 1:
                tail((it - 2) // 2)


# ---------------------------------------------------------------------------
# host-side prep / unshard
# ---------------------------------------------------------------------------

def plan_shards(batch_i32, B, ncores, tblk=TBLK, gblk=GBLK):
    cnt = np.bincount(batch_i32, minlength=B).astype(np.int64)
    starts = np.concatenate([[0], np.cumsum(cnt)])
    N = int(starts[-1])
    bounds = [0]
    for c in range(1, ncores):
        target = N * c // ncores
        g = int(np.searchsorted(starts, target))
        g = max(bounds[-1], min(g, B))
        bounds.append(g)
    bounds.append(B)
    cap = tblk * 128
    plans = []
    for c in range(ncores):
        glo, ghi = bounds[c], bounds[c + 1]
        blocks, cur, cur_nodes = [], [], 0
        for g in range(glo, ghi):
            n_g = int(cnt[g])
            assert n_g <= cap, f"graph {g} has {n_g} nodes > block capacity"
            if cur and (cur_nodes + n_g > cap or len(cur) >= gblk):
                blocks.append(cur)
                cur, cur_nodes = [], 0
            cur.append((g, int(starts[g]), n_g))
            cur_nodes += n_g
        if cur:
            blocks.append(cur)
        plans.append(blocks)
    NB = max(len(p) for p in plans)
    return plans, NB


def prep_core(x, plan, cfg):
    NB, TBLKc, GBLKc = cfg.NB, cfg.TBLK, cfg.GBLK
    xs = np.zeros((cfg.NNODES, D), np.float32)
    sl = np.zeros((cfg.NTILES * 128, GBLKc), NPBF)
    recip = np.zeros((NB, GBLKc), np.float32)
    meta = []
    for bi, blkg in enumerate(plan):
        pos = bi * TBLKc * 128
        for slot, (g, s, n_g) in enumerate(blkg):
            xs[pos:pos + n_g] = x[s:s + n_g]
            sl[pos:pos + n_g, slot] = NPBF(2.0)   # x2 = 2*y2 folded here
            recip[bi, slot] = 1.0 / max(n_g, 1)
            meta.append((bi, slot, g))
            pos += n_g
    xt = np.ascontiguousarray(xs.astype(NPBF).T)
    sl_packed = np.ascontiguousarray(
        sl.reshape(cfg.NTILES, 128, GBLKc).transpose(1, 0, 2).reshape(128, cfg.NTILES * GBLKc))
    recip_b = np.ascontiguousarray(
        np.broadcast_to(recip.reshape(1, NB * GBLKc), (128, NB * GBLKc)))
    return {"xt": xt, "sl": sl_packed, "recip": recip_b}, meta


def prep_consts(Wm, fc1_w, fc1_b, fc2_w, fc2_b):
    assert np.allclose(np.asarray(fc2_b, np.float32), 0.0), \
        "nonzero fc2_b not supported by this kernel build"
    fc1t = np.ascontiguousarray(np.asarray(fc1_w, np.float32).T.astype(NPBF))
    # fc2e[:, j*128:(j+1)*128] has fc2e[32j+u, f] = fc2_w[f, u], zero elsewhere
    fc2e = np.zeros((128, 512), NPBF)
    f2 = np.asarray(fc2_w, np.float32).astype(NPBF)   # [128 f, 32 u]
    for j in range(4):
        fc2e[32 * j:32 * j + 32, j * 128:(j + 1) * 128] = f2.T
    b1 = np.tile(np.asarray(fc1_b, np.float32), 4).reshape(128, 1)
    wm = np.ascontiguousarray(np.asarray(Wm, np.float32))
    ident = np.eye(128, dtype=NPBF)
    return {"fc1t": fc1t, "fc2e": fc2e, "wm": wm,
            "b1": np.ascontiguousarray(b1), "ident": ident}


def unshard(outTs, metas, B, cfg):
    out = np.zeros((B, D), np.float32)
    for outT, meta in zip(outTs, metas):
        cols = [bi * cfg.GBLK + slot for (bi, slot, g) in meta]
        gs = [g for (bi, slot, g) in meta]
        out[gs] = outT[:, cols].T
    return out


# ---------------------------------------------------------------------------
# top-level entry
# ---------------------------------------------------------------------------

_CACHE = {}


def _get_program(NB):
    key = (NB, TBLK, GBLK)
    if key not in _CACHE:
        nc = bacc.Bacc("TRN2", target_bir_lowering=False, debug=False,
                       num_devices=NCORES)
        cfg = Cfg(NB)
        io = declare_io(nc, cfg)
        with tile.TileContext(nc) as tc:
            build(tc, io, cfg)
        nc.compile()
        _CACHE[key] = (nc, cfg)
    return _CACHE[key]


def _run(inputs, trace=False):
    x = np.asarray(inputs["x"], np.float32)
    batch = np.asarray(inputs["batch"]).astype(np.int32)
    B = int(np.asarray(inputs["size"]))
    plans, NB = plan_shards(batch, B, NCORES)
    nc, cfg = _get_program(NB)
    consts = prep_consts(inputs["Wm"], inputs["fc1_w"], inputs["fc1_b"],
                         inputs["fc2_w"], inputs["fc2_b"])
    in_maps, metas = [], []
    for c in range(NCORES):
        core_in, meta = prep_core(x, plans[c], cfg)
        core_in.update(consts)
        in_maps.append(core_in)
        metas.append(meta)
    res = run_bass_kernel_spmd(nc, in_maps, core_ids=list(range(NCORES)),
                               trace=trace)
    outTs = [res.results[c]["outT"] for c in range(NCORES)]
    out = unshard(outTs, metas, B, cfg)
    return out, res


def kernel(**inputs):
    out, _ = _run(inputs, trace=False)
    return out
